# revision 21
# baseline (speedup 1.0000x reference)
"""Distributed Trainium2 Bass kernel for nn_Attention_69973607186925.

Multi-head attention (N=288 tokens, B=64 batch, C=1024, H=16 heads) with a
prompt-structured mask, data-parallel over batch across 8 NeuronCores
(8 batches = 128 heads per core, zero collectives).

Per-core dataflow (all matmuls bf16 -> f32 PSUM):
  phase A: QKV projections. q/k produced TRANSPOSED [c, token] (c on
           partitions) as scores operands; v produced NATURAL [token, c]
           as the PV stationary operand. Weights streamed, x resident.
  phase B: per (batch, head-pair): scoresT[m,n] = kT.T @ qT on the PE
           (keys m on partitions, queries n free), exp on ACT (scale 1/8
           folded in), prompt mask applied as a binary multiply on the
           first 32 key rows, PV = v.T @ expT accumulated into a
           pair-shared PSUM bank (odd head at column-position 64), column
           sums via ones-vector matmuls, reciprocal + cross-partition
           broadcast of 1/sum via a replicating SBUF->SBUF DMA, fused
           normalize-multiply into the transposed output [c, token].
  phase C: output projection from outT, bias added via per-partition
           scalar on the PSUM->SBUF copy, DMA to DRAM [1024, 2304].

Host side: shard batch, pre-transpose/pre-cast inputs (free), gather and
re-transpose the 8 per-core outputs.
"""

import sys

if "/opt/trn_rl_repo" not in sys.path:
    sys.path.insert(0, "/opt/trn_rl_repo")

import numpy as np
import ml_dtypes

import concourse.bass as bass
import concourse.mybir as mybir
import concourse.tile as tile
from concourse.bass_utils import run_bass_kernel_spmd

BF16 = mybir.dt.bfloat16
F32 = mybir.dt.float32

N = 288          # tokens per batch
BL = 8           # batches per core
C = 1024
H = 16           # heads per batch
HD = 64          # head dim
T = BL * N       # tokens per core (2304)
CT = C // 128    # c tiles (8)
NCH = T // N     # token chunks of 288 (8)
SCALE = HD ** -0.5
M_TILES = [(0, 128), (128, 128), (256, 32)]  # key tiles per batch


def _install_tile_drain_patch():
    """walrus in this container accepts only ONE semaphore wait per sync
    (SP) engine instruction; TileContext's final drain carries one wait
    per live semaphore.  Split them across single-wait nops (same engine,
    program order) before the drain."""
    from concourse.vector_clock import ScopedClock

    if getattr(tile.TileContext, "_drain_patch_installed", False):
        return

    def _drain_and_barrier_chunked(self, tick_clock, wait_clock):
        nc = self.nc
        collector = nc.sync.nop(nofuse=True, hint="drain_wait_collector")
        wait_clock.add_sem_waits(
            collector.ins, ScopedClock({None: tick_clock.global_clock})
        )
        si = collector.ins.sync_info
        waits = list(si.on_wait) if si and si.on_wait else []
        if len(waits) > 1:
            si.on_wait = waits[:1]
            for w in waits[1:]:
                extra = nc.sync.nop(nofuse=True, hint="drain_wait_chunk")
                esi = extra.ins.sync_info
                if esi is None:
                    extra.ins.sync_info = mybir.SyncInfo(on_wait=[w], on_update=[])
                else:
                    esi.on_wait = (esi.on_wait or []) + [w]
        nc.sync.drain()

        nc.all_engine_barrier()
        assert self.sems is not None
        popped = nc._tile_sem_poison_stack.pop()
        assert popped is self._sem_poison
        nc.clear_and_free_semaphores(list(self.sems.allocated().values()))
        nc.all_engine_barrier()

    tile.TileContext._drain_and_barrier = _drain_and_barrier_chunked
    tile.TileContext._drain_patch_installed = True


def _split_multi_waits(nc):
    """walrus in this container accepts only one semaphore wait per
    instruction.  For any instruction carrying N>1 waits, hoist N-1 of
    them onto same-engine NoOps placed immediately before it — engine
    program order makes this equivalent."""
    for fn in nc.m.functions:
        for blk in fn.blocks:
            insts = blk.instructions
            out = []
            changed = False
            for inst in insts:
                si = inst.sync_info
                if si is not None and si.on_wait and len(si.on_wait) > 1:
                    waits = list(si.on_wait)
                    for idx, w in enumerate(waits[:-1]):
                        out.append(
                            mybir.InstNoOp(
                                name=f"{inst.name}-hw{idx}",
                                engine=inst.engine,
                                ins=[],
                                outs=[],
                                bass_nofuse=True,
                                sync_info=mybir.SyncInfo(on_wait=[w], on_update=[]),
                            )
                        )
                    si.on_wait = [waits[-1]]
                    changed = True
                out.append(inst)
            if changed:
                insts[:] = out


def _build_nc(split_waits=True):
    _install_tile_drain_patch()
    nc = bass.Bass()

    xt_ext = nc.declare_dram_parameter("xt", [C, T], BF16, isOutput=False)
    wqkt_ext = nc.declare_dram_parameter("wqkt", [C, 2 * C], BF16, isOutput=False)
    wvt_ext = nc.declare_dram_parameter("wvt", [C, C], BF16, isOutput=False)
    wpt_ext = nc.declare_dram_parameter("wpt", [C, C], BF16, isOutput=False)
    bv_ext = nc.declare_dram_parameter("bv", [1, C], BF16, isOutput=False)
    bqk_ext = nc.declare_dram_parameter("bqk", [128, 16], F32, isOutput=False)
    bp_ext = nc.declare_dram_parameter("bp", [128, CT], F32, isOutput=False)
    mask_ext = nc.declare_dram_parameter("binmask", [32, N], BF16, isOutput=False)
    sel2_ext = nc.declare_dram_parameter("sel2", [2, 128], BF16, isOutput=False)
    out_ext = nc.declare_dram_parameter("out", [C, T], F32, isOutput=True)

    xt_r = xt_ext.rearrange("(o p) t -> p o t", p=128)
    wqkt_r = wqkt_ext.rearrange("(o p) j -> p o j", p=128)
    wvt_r = wvt_ext.rearrange("(o p) j -> p o j", p=128)
    wpt_r = wpt_ext.rearrange("(o p) j -> p o j", p=128)
    out_r = out_ext.rearrange("(o p) t -> p o t", p=128)

    with tile.TileContext(nc) as tc:
        with (
            tc.tile_pool(name="persist", bufs=1) as persist,
            tc.tile_pool(name="consts", bufs=1) as consts,
            # scores PSUM + exp staging live at the outer level: the first
            # eight pairs' scores/exp are emitted inside phase A (ACT is idle
            # there), giving the exp pipeline an 8-pair lead before phase B
            tc.tile_pool(name="psS", bufs=2, space="PSUM") as pss_pool,
            tc.tile_pool(name="expt", bufs=2) as expt_pool,
        ):
            qt_sb = persist.tile([128, CT, T], BF16, tag="qt")
            kt_sb = persist.tile([128, CT, T], BF16, tag="kt")
            v_sb = persist.tile([128, BL, 2, C], BF16, tag="v")
            v2_sb = persist.tile([128, 2, C], BF16, tag="v2")

            bqk_sb = consts.tile([128, 16], F32, tag="bqk")
            bp_sb = consts.tile([128, CT], F32, tag="bp")
            bv_sb = consts.tile([1, C], BF16, tag="bv")
            mask_sb = consts.tile([32, N], BF16, tag="binmask")
            ones_sb = consts.tile([128, 32], BF16, tag="ones")
            zbias_sb = consts.tile([128, 1], F32, tag="zbias")
            onesr_sb = consts.tile([1, 128], BF16, tag="onesr")
            sel2_sb = consts.tile([2, 128], BF16, tag="sel2")
            nc.sync.dma_start(out=sel2_sb[:], in_=sel2_ext[:])
            nc.sync.dma_start(out=bqk_sb[:], in_=bqk_ext[:])
            nc.sync.dma_start(out=bp_sb[:], in_=bp_ext[:])
            nc.sync.dma_start(out=bv_sb[:], in_=bv_ext[:])
            nc.sync.dma_start(out=mask_sb[:], in_=mask_ext[:])
            nc.vector.memset(ones_sb[:], 1.0)
            nc.vector.memset(zbias_sb[:], 0.0)
            nc.vector.memset(onesr_sb[:], 1.0)

            # per-half expt tiles, keyed for lagged use by PV/sums
            expt_tiles = {}   # half_id -> [expt_mt0, expt_mt1, expt_mt2]

            def sc_mt(i, mt):
                b, p = i // 8, i % 8
                pp = p % 4
                o = p
                moff, msize = M_TILES[mt]
                mb = (b % 4) * 32 if mt == 2 else 0
                if p % 4 == 0 and mt == 0:
                    expt_tiles[i // 4] = [
                        expt_pool.tile(
                            [128, 8, N], BF16, tag=f"expt{m}", name=f"expt{m}"
                        )
                        for m in range(3)
                    ]
                if mt == 0:
                    sc_mt.ps_s = pss_pool.tile(
                        [128, 2, 512], F32, tag="ps_s", name="ps_s"
                    )
                ps_s = sc_mt.ps_s
                for hh in range(2):
                    rb = 64 * hh
                    nc.tensor.matmul(
                        ps_s[mb : mb + msize, hh, 0:N],
                        lhsT=kt_sb[
                            rb : rb + 64,
                            o,
                            b * N + moff : b * N + moff + msize,
                        ],
                        rhs=qt_sb[rb : rb + 64, o, b * N : (b + 1) * N],
                        start=True,
                        stop=True,
                        tile_position=(rb, mb) if mt == 2 else None,
                    )
                nc.scalar.activation(
                    out=expt_tiles[i // 4][mt][
                        mb : mb + msize, 2 * pp : 2 * pp + 2, :
                    ],
                    in_=ps_s[mb : mb + msize, :, 0:N],
                    func=mybir.ActivationFunctionType.Exp,
                    bias=zbias_sb[0:msize, 0:1],
                    scale=SCALE,
                )

            def mask_op(i):
                pp = (i % 8) % 4
                nc.vector.tensor_tensor(
                    expt_tiles[i // 4][0][0:32, 2 * pp : 2 * pp + 2, :],
                    expt_tiles[i // 4][0][0:32, 2 * pp : 2 * pp + 2, :],
                    mask_sb[:, None, :].to_broadcast((32, 2, N)),
                    mybir.AluOpType.mult,
                )

            PREFIX = 8  # pairs whose scores/exp are emitted inside phase A

            # ---------------- phase A: QKV projections ----------------
            with (
                tc.tile_pool(name="xa", bufs=1) as xa_pool,
                tc.tile_pool(name="wa", bufs=2) as wa_pool,
                tc.tile_pool(name="psA", bufs=2, space="PSUM") as psa_pool,
                tc.tile_pool(name="psAv", bufs=2, space="PSUM") as psav_pool,
            ):
                # prefetch the first weight tile BEFORE the bulky xt loads so
                # the first matmul doesn't queue ~20us behind 4.7MB of x DMA
                w_first = wa_pool.tile([128, CT, 128], BF16, tag="wqk", name="w_first")
                nc.sync.dma_start(out=w_first[:], in_=wqkt_r[:, :, 0:128])

                xt_sb = xa_pool.tile([128, CT, T], BF16, tag="xt")
                for o in range(CT):
                    nc.sync.dma_start(out=xt_sb[:, o, :], in_=xt_r[:, o, :])

                # q then k, transposed layout [cq, t]
                for proj in range(2):
                    dst = qt_sb if proj == 0 else kt_sb
                    for o in range(CT):
                        if proj == 0 and o == 0:
                            w_sb = w_first
                        else:
                            w_sb = wa_pool.tile(
                                [128, CT, 128], BF16, tag="wqk", name="w_sb"
                            )
                            j0 = proj * C + o * 128
                            nc.sync.dma_start(
                                out=w_sb[:], in_=wqkt_r[:, :, j0 : j0 + 128]
                            )
                        for c0 in range(0, T, 512):
                            csz = min(512, T - c0)
                            ps = psa_pool.tile([128, 512], F32, tag="psqk")
                            for kk in range(CT):
                                nc.tensor.matmul(
                                    ps[:, 0:csz],
                                    lhsT=w_sb[:, kk, :],
                                    rhs=xt_sb[:, kk, c0 : c0 + csz],
                                    start=(kk == 0),
                                    stop=(kk == CT - 1),
                                )
                            nc.vector.tensor_scalar(
                                out=dst[:, o, c0 : c0 + csz],
                                in0=ps[:, 0:csz],
                                scalar1=bqk_sb[:, proj * 8 + o : proj * 8 + o + 1],
                                scalar2=None,
                                op0=mybir.AluOpType.add,
                            )

                # scores+exp+mask for the first PREFIX pairs: their matmuls
                # slide into qk-tail PE bubbles, their exps run on the
                # otherwise-idle ACT, and their PV/sums become dense PE
                # filler for the phase-A -> phase-B transition
                for i in range(PREFIX):
                    for mt in range(3):
                        sc_mt(i, mt)
                    mask_op(i)

                # contiguous staging of the 32-token mt2 tails, 4 batches
                # per 128-wide group (walrus: stationary AP needs 1 free dim)
                xg2_sb = xa_pool.tile([128, CT, 2, 128], BF16, tag="xg2")
                for kk in range(CT):
                    for g in range(2):
                        nc.vector.tensor_copy(
                            xg2_sb[:, kk, g, :],
                            xt_sb[:, kk, :].rearrange("p (b n) -> p b n", n=N)[
                                :, 4 * g : 4 * g + 4, 256:288
                            ],
                        )

                # v, natural layout [token, cv]
                for ch in range(2):
                    wv_sb = wa_pool.tile([128, CT, 512], BF16, tag="wv")
                    nc.sync.dma_start(
                        out=wv_sb[:], in_=wvt_r[:, :, ch * 512 : (ch + 1) * 512]
                    )
                    for b in range(BL):
                        for mt, (moff, msize) in enumerate(M_TILES[:2]):
                            t0 = b * N + moff
                            ps = psav_pool.tile([128, 512], F32, tag="psv")
                            for kk in range(CT):
                                nc.tensor.matmul(
                                    ps[:msize, :],
                                    lhsT=xt_sb[:, kk, t0 : t0 + msize],
                                    rhs=wv_sb[:, kk, :],
                                    start=(kk == 0),
                                    stop=False,
                                )
                            # bias row via rank-1 matmul (ones ⊗ bv)
                            nc.tensor.matmul(
                                ps[:msize, :],
                                lhsT=onesr_sb[0:1, 0:msize],
                                rhs=bv_sb[0:1, ch * 512 : (ch + 1) * 512],
                                start=False,
                                stop=True,
                            )
                            nc.scalar.copy(
                                out=v_sb[0:msize, b, mt, ch * 512 : (ch + 1) * 512],
                                in_=ps[:msize, :],
                            )
                    # mt2 (32-token tails): 4 batches packed on partitions
                    for g in range(2):
                        ps = psav_pool.tile([128, 512], F32, tag="psv")
                        for kk in range(CT):
                            nc.tensor.matmul(
                                ps[:],
                                lhsT=xg2_sb[:, kk, g, :],
                                rhs=wv_sb[:, kk, :],
                                start=(kk == 0),
                                stop=False,
                            )
                        nc.tensor.matmul(
                            ps[:],
                            lhsT=onesr_sb[0:1, 0:128],
                            rhs=bv_sb[0:1, ch * 512 : (ch + 1) * 512],
                            start=False,
                            stop=True,
                        )
                        for jj in range(4):
                            nc.scalar.copy(
                                out=v2_sb[
                                    32 * jj : 32 * jj + 32,
                                    g,
                                    ch * 512 : (ch + 1) * 512,
                                ],
                                in_=ps[32 * jj : 32 * jj + 32, :],
                            )

            # ---------------- phases B+C (global pair pipeline) ----------------
            # One "slot" per head-pair i (64 total).  At slot s we emit:
            #   scores+exp+mask for pair s (3 mt rounds, exp after each),
            #   PV for pair s-1 / sums for pair s-1 (fills the exp gaps),
            #   reciprocal for batch s//8-1 when s%8==0,
            #   bcast+normalize for pair s-9 (previous batch),
            #   out-projection chunk for pair s-10.
            # The 1-slot/9-slot lags keep the PE instruction stream dense so
            # the HAM clock gate stays at 8/8 (2.4 GHz) through phase B.
            with (
                tc.tile_pool(name="wpt", bufs=1) as wpt_pool,
                tc.tile_pool(name="outt", bufs=2) as outt_pool,
                tc.tile_pool(name="yc", bufs=3) as yc_pool,
                tc.tile_pool(name="pvs", bufs=2) as pvs_pool,
                tc.tile_pool(name="sums", bufs=2) as sums_pool,
                tc.tile_pool(name="densep", bufs=10) as densep_pool,
                tc.tile_pool(name="psPV", bufs=1, space="PSUM") as pspv_pool,
                tc.tile_pool(name="psSum", bufs=1, space="PSUM") as pssum_pool,
                tc.tile_pool(name="psBC", bufs=1, space="PSUM") as psbc_pool,
                tc.tile_pool(name="psC", bufs=1, space="PSUM") as psc_pool,
            ):
                wpt_sb = wpt_pool.tile([128, CT, C], BF16, tag="wpt")
                for kk in range(CT):
                    nc.sync.dma_start(out=wpt_sb[:, kk, :], in_=wpt_r[:, kk, :])

                # per-batch staging, keyed for lagged use
                pvstage_t = {}    # batch -> tile
                sums_t = {}       # batch -> (sums_sb, sums_sr)
                dense_t = {}      # batch -> tile
                denseb_t = {}     # batch -> tile
                outt_t = {}       # batch -> tile

                def pv_block(i):
                    b, p = i // 8, i % 8
                    expt = expt_tiles[i // 4]
                    if p == 0:
                        pvstage_t[b] = pvs_pool.tile(
                            [128, 8, N], BF16, tag="pvstage", name="pvstage"
                        )
                    ps_pv = pspv_pool.tile([128, N], F32, tag="ps_pv")
                    pv_block.ps_pv = ps_pv
                    # mt-major: the two heads' matmuls hit distinct 64-col
                    # array strips (col tiling) and run concurrently.  Only
                    # the very first matmul carries start=True — its whole-
                    # bank has_written clear covers both head regions.
                    for mt, (moff, msize) in enumerate(M_TILES):
                        mb = (b % 4) * 32 if mt == 2 else 0
                        for hh in range(2):
                            h = 2 * p + hh
                            slot = h % 8
                            lhsT_v = (
                                v_sb[0:msize, b, mt, h * 64 : h * 64 + 64]
                                if mt < 2
                                else v2_sb[
                                    mb : mb + 32, b // 4, h * 64 : h * 64 + 64
                                ]
                            )
                            nc.tensor.matmul(
                                ps_pv[64 * hh : 64 * hh + 64, :],
                                lhsT=lhsT_v,
                                rhs=expt[mt][mb : mb + msize, slot, :],
                                start=(mt == 0),
                                stop=(mt == 2),
                                skip_group_check=True,
                                tile_position=((mb, 64 * hh) if mt == 2 else None),
                            )

                def sums_block(i):
                    b, p = i // 8, i % 8
                    expt = expt_tiles[i // 4]
                    if p == 0:
                        sums_sb = sums_pool.tile(
                            [128, 2, N], F32, tag="sums", name="sums"
                        )
                        sums_t[b] = (
                            sums_sb,
                            sums_sb.rearrange("(a c) s n -> a c s n", c=32),
                        )
                        dense_t[b] = sums_pool.tile(
                            [16, N], F32, tag="dense", name="dense"
                        )
                        denseb_t[b] = sums_pool.tile(
                            [16, N], BF16, tag="denseb", name="denseb"
                        )
                    sums_sb, sums_sr = sums_t[b]
                    ps_sm = pssum_pool.tile([128, N], F32, tag="ps_sm")
                    for mt, (moff, msize) in enumerate(M_TILES):
                        mb = (b % 4) * 32 if mt == 2 else 0
                        for hh in range(2):
                            h = 2 * p + hh
                            slot = h % 8
                            # ones [m, 32]: the column sum lands replicated on
                            # 32 partition rows so the later [0:33] copy reads
                            # no uninit PSUM
                            nc.tensor.matmul(
                                ps_sm[32 * hh : 32 * hh + 32, :],
                                lhsT=ones_sb[mb : mb + msize, :],
                                rhs=expt[mt][mb : mb + msize, slot, :],
                                start=(mt == 0),
                                stop=(mt == 2),
                                skip_group_check=True,
                                tile_position=((mb, 32 * hh) if mt == 2 else None),
                            )
                    # stage PV out of PSUM on the DVE (ACT is exp-bound)
                    nc.vector.tensor_copy(
                        pvstage_t[b][:, p, :], pv_block.ps_pv[:]
                    )
                    nc.vector.tensor_copy(sums_sb[0:33, p % 2, :], ps_sm[0:33, :])
                    nc.sync.dma_start(
                        out=dense_t[b][2 * p : 2 * p + 2, :],
                        in_=sums_sr[0:2, 0, p % 2, :],
                    )

                dp_t = {}  # pair -> [2, N] bf16 reciprocal row pair

                def recip_block(b):
                    nc.vector.reciprocal(out=dense_t[b][:], in_=dense_t[b][:])
                    nc.vector.tensor_copy(denseb_t[b][:], dense_t[b][:])
                    # prefetch ALL eight pairs' dp rows now so the
                    # bcast-matmul -> normalize chain never waits on DMA
                    # latency mid-slot (batch boundaries, kernel tail)
                    for p in range(8):
                        dp = densep_pool.tile([2, N], BF16, tag="dp", name="dp")
                        nc.sync.dma_start(
                            out=dp[:], in_=denseb_t[b][2 * p : 2 * p + 2, :]
                        )
                        dp_t[8 * b + p] = dp

                def bcast_norm_block(i):
                    b, p = i // 8, i % 8
                    if p == 0:
                        outt_t[b] = outt_pool.tile(
                            [128, CT, N], BF16, tag="outt_b", name="outt_b"
                        )
                    # broadcast via selector matmul: psbc[P,n] = dp[P//64,n]
                    psbc = psbc_pool.tile([128, N], F32, tag="psbc")
                    nc.tensor.matmul(
                        psbc[:],
                        lhsT=sel2_sb[:],
                        rhs=dp_t.pop(i)[:],
                        start=True,
                        stop=True,
                    )
                    nc.vector.tensor_tensor(
                        outt_t[b][:, p, :],
                        pvstage_t[b][:, p, :],
                        psbc[:],
                        mybir.AluOpType.mult,
                    )

                def proj_block(i):
                    b, o = i // 8, i % 8
                    ps = psc_pool.tile([128, N], F32, tag="psy", name="psy")
                    for kk in range(CT):
                        nc.tensor.matmul(
                            ps[:],
                            lhsT=wpt_sb[:, kk, o * 128 : (o + 1) * 128],
                            rhs=outt_t[b][:, kk, :],
                            start=(kk == 0),
                            stop=(kk == CT - 1),
                        )
                    y_sb = yc_pool.tile([128, N], F32, tag="y", name="y")
                    nc.vector.tensor_scalar(
                        out=y_sb[:],
                        in0=ps[:],
                        scalar1=bp_sb[:, o : o + 1],
                        scalar2=None,
                        op0=mybir.AluOpType.add,
                    )
                    nc.sync.dma_start(
                        out=out_r[:, o, b * N : (b + 1) * N], in_=y_sb[:]
                    )

                # proj lags 17 slots: chunk (b, o) reads ALL eight outt(b)
                # slices, and the last one is only EMITTED at slot 8b+16 —
                # emitting a read before its producing write is emitted gives
                # stale data (Tile orders reads only against prior writes).
                NP = BL * 8  # 64 pairs
                for s in range(NP + 17):
                    if s < NP:
                        if s >= PREFIX:
                            sc_mt(s, 0)
                        if s >= 1:
                            pv_block(s - 1)
                        if s >= PREFIX:
                            sc_mt(s, 1)
                        if s >= 1:
                            sums_block(s - 1)
                        if s >= 8 and s % 8 == 0:
                            recip_block(s // 8 - 1)
                        if s >= PREFIX:
                            sc_mt(s, 2)
                            mask_op(s)
                        if s >= 9:
                            bcast_norm_block(s - 9)
                        if s >= 17:
                            proj_block(s - 17)
                    else:
                        if s - 1 < NP:
                            pv_block(s - 1)
                            sums_block(s - 1)
                        if s % 8 == 0 and s // 8 - 1 < BL:
                            recip_block(s // 8 - 1)
                        if s - 9 < NP:
                            bcast_norm_block(s - 9)
                        if s - 17 >= 0:
                            proj_block(s - 17)

    if split_waits:
        _split_multi_waits(nc)
    return nc


_NC_CACHE = None


def _get_nc():
    global _NC_CACHE
    if _NC_CACHE is None:
        _NC_CACHE = _build_nc()
    return _NC_CACHE


def _host_inputs(x, Wqkv, bqkv, Wproj, bproj):
    bf16 = ml_dtypes.bfloat16
    shared = {}
    shared["wqkt"] = np.ascontiguousarray(Wqkv[: 2 * C].T).astype(bf16)
    shared["wvt"] = np.ascontiguousarray(Wqkv[2 * C :].T).astype(bf16)
    shared["wpt"] = np.ascontiguousarray(Wproj.T).astype(bf16)
    shared["bv"] = bqkv[2 * C :].reshape(1, C).astype(bf16)
    shared["bqk"] = np.ascontiguousarray(
        bqkv[: 2 * C].reshape(2, 8, 128).transpose(2, 0, 1).reshape(128, 16)
    ).astype(np.float32)
    shared["bp"] = np.ascontiguousarray(bproj.reshape(CT, 128).T).astype(np.float32)
    sel2 = np.zeros((2, 128), bf16)
    sel2[0, 0:64] = 1.0
    sel2[1, 64:128] = 1.0
    shared["sel2"] = sel2
    m_ = np.arange(32)[:, None]
    n_ = np.arange(N)[None, :]
    shared["binmask"] = ((n_ < 32) & (n_ >= 4 * (m_ // 4))).astype(bf16)

    in_maps = []
    for i in range(8):
        xc = x[:, i * BL : (i + 1) * BL, :]  # (N, BL, C)
        xt = np.ascontiguousarray(xc.transpose(2, 1, 0).reshape(C, T)).astype(bf16)
        m = dict(shared)
        m["xt"] = xt
        in_maps.append(m)
    return in_maps


def kernel(x, Wqkv, bqkv, Wproj, bproj):
    x = np.asarray(x, dtype=np.float32)
    Wqkv = np.asarray(Wqkv, dtype=np.float32)
    bqkv = np.asarray(bqkv, dtype=np.float32)
    Wproj = np.asarray(Wproj, dtype=np.float32)
    bproj = np.asarray(bproj, dtype=np.float32)

    nc = _get_nc()
    in_maps = _host_inputs(x, Wqkv, bqkv, Wproj, bproj)
    res = run_bass_kernel_spmd(nc, in_maps, core_ids=list(range(8)))

    full = np.empty((N, 64, C), dtype=np.float32)
    for i in range(8):
        yT = np.asarray(res.results[i]["out"], dtype=np.float32)  # [C, T]
        full[:, i * BL : (i + 1) * BL, :] = yT.reshape(C, BL, N).transpose(2, 1, 0)
    return full



# revision 22
# speedup vs baseline: 1.2001x; 1.2001x over previous
"""Distributed Trainium2 Bass kernel for nn_Attention_69973607186925.

Multi-head attention (N=288 tokens, B=64 batch, C=1024, H=16 heads) with a
prompt-structured mask, data-parallel over batch across 8 NeuronCores
(8 batches = 128 heads per core, zero collectives).

Per-core dataflow (all matmuls bf16 -> f32 PSUM):
  phase A: QKV projections. q/k produced TRANSPOSED [c, token] (c on
           partitions) as scores operands; v produced NATURAL [token, c]
           as the PV stationary operand. Weights streamed, x resident.
  phase B: per (batch, head-pair): scoresT[m,n] = kT.T @ qT on the PE
           (keys m on partitions, queries n free), exp on ACT (scale 1/8
           folded in), prompt mask applied as a binary multiply on the
           first 32 key rows, PV = v.T @ expT accumulated into a
           pair-shared PSUM bank (odd head at column-position 64), column
           sums via ones-vector matmuls, reciprocal + cross-partition
           broadcast of 1/sum via a replicating SBUF->SBUF DMA, fused
           normalize-multiply into the transposed output [c, token].
  phase C: output projection from outT, bias added via per-partition
           scalar on the PSUM->SBUF copy, DMA to DRAM [1024, 2304].

Host side: shard batch, pre-transpose/pre-cast inputs (free), gather and
re-transpose the 8 per-core outputs.
"""

import sys

if "/opt/trn_rl_repo" not in sys.path:
    sys.path.insert(0, "/opt/trn_rl_repo")

import numpy as np
import ml_dtypes

import concourse.bass as bass
import concourse.mybir as mybir
import concourse.tile as tile
from concourse.bass_utils import run_bass_kernel_spmd

BF16 = mybir.dt.bfloat16
F32 = mybir.dt.float32

N = 288          # tokens per batch
BL = 8           # batches per core
C = 1024
H = 16           # heads per batch
HD = 64          # head dim
T = BL * N       # tokens per core (2304)
CT = C // 128    # c tiles (8)
NCH = T // N     # token chunks of 288 (8)
SCALE = HD ** -0.5
M_TILES = [(0, 128), (128, 128), (256, 32)]  # key tiles per batch


def _install_tile_drain_patch():
    """walrus in this container accepts only ONE semaphore wait per sync
    (SP) engine instruction; TileContext's final drain carries one wait
    per live semaphore.  Split them across single-wait nops (same engine,
    program order) before the drain."""
    from concourse.vector_clock import ScopedClock

    if getattr(tile.TileContext, "_drain_patch_installed", False):
        return

    def _drain_and_barrier_chunked(self, tick_clock, wait_clock):
        nc = self.nc
        collector = nc.sync.nop(nofuse=True, hint="drain_wait_collector")
        wait_clock.add_sem_waits(
            collector.ins, ScopedClock({None: tick_clock.global_clock})
        )
        si = collector.ins.sync_info
        waits = list(si.on_wait) if si and si.on_wait else []
        if len(waits) > 1:
            si.on_wait = waits[:1]
            for w in waits[1:]:
                extra = nc.sync.nop(nofuse=True, hint="drain_wait_chunk")
                esi = extra.ins.sync_info
                if esi is None:
                    extra.ins.sync_info = mybir.SyncInfo(on_wait=[w], on_update=[])
                else:
                    esi.on_wait = (esi.on_wait or []) + [w]
        nc.sync.drain()

        nc.all_engine_barrier()
        assert self.sems is not None
        popped = nc._tile_sem_poison_stack.pop()
        assert popped is self._sem_poison
        nc.clear_and_free_semaphores(list(self.sems.allocated().values()))
        nc.all_engine_barrier()

    tile.TileContext._drain_and_barrier = _drain_and_barrier_chunked
    tile.TileContext._drain_patch_installed = True


def _split_multi_waits(nc):
    """walrus in this container accepts only one semaphore wait per
    instruction.  For any instruction carrying N>1 waits, hoist N-1 of
    them onto same-engine NoOps placed immediately before it — engine
    program order makes this equivalent."""
    for fn in nc.m.functions:
        for blk in fn.blocks:
            insts = blk.instructions
            out = []
            changed = False
            for inst in insts:
                si = inst.sync_info
                if si is not None and si.on_wait and len(si.on_wait) > 1:
                    waits = list(si.on_wait)
                    for idx, w in enumerate(waits[:-1]):
                        out.append(
                            mybir.InstNoOp(
                                name=f"{inst.name}-hw{idx}",
                                engine=inst.engine,
                                ins=[],
                                outs=[],
                                bass_nofuse=True,
                                sync_info=mybir.SyncInfo(on_wait=[w], on_update=[]),
                            )
                        )
                    si.on_wait = [waits[-1]]
                    changed = True
                out.append(inst)
            if changed:
                insts[:] = out


def _build_nc(split_waits=True):
    _install_tile_drain_patch()
    nc = bass.Bass()

    xt_ext = nc.declare_dram_parameter("xt", [C, T], BF16, isOutput=False)
    wqkt_ext = nc.declare_dram_parameter("wqkt", [C, 2 * C], BF16, isOutput=False)
    wvt_ext = nc.declare_dram_parameter("wvt", [C, C], BF16, isOutput=False)
    wpt_ext = nc.declare_dram_parameter("wpt", [C, C], BF16, isOutput=False)
    bv_ext = nc.declare_dram_parameter("bv", [1, C], BF16, isOutput=False)
    bqk_ext = nc.declare_dram_parameter("bqk", [128, 16], F32, isOutput=False)
    bp_ext = nc.declare_dram_parameter("bp", [128, CT], F32, isOutput=False)
    mask_ext = nc.declare_dram_parameter("binmask", [32, N], BF16, isOutput=False)
    sel2_ext = nc.declare_dram_parameter("sel2", [2, 128], BF16, isOutput=False)
    out_ext = nc.declare_dram_parameter("out", [C, T], F32, isOutput=True)

    xt_r = xt_ext.rearrange("(o p) t -> p o t", p=128)
    wqkt_r = wqkt_ext.rearrange("(o p) j -> p o j", p=128)
    wvt_r = wvt_ext.rearrange("(o p) j -> p o j", p=128)
    wpt_r = wpt_ext.rearrange("(o p) j -> p o j", p=128)
    out_r = out_ext.rearrange("(o p) t -> p o t", p=128)

    with tile.TileContext(nc) as tc:
        with (
            tc.tile_pool(name="persist", bufs=1) as persist,
            tc.tile_pool(name="consts", bufs=1) as consts,
        ):
            qt_sb = persist.tile([128, CT, T], BF16, tag="qt")
            kt_sb = persist.tile([128, CT, T], BF16, tag="kt")
            v_sb = persist.tile([128, BL, 2, C], BF16, tag="v")
            v2_sb = persist.tile([128, 2, C], BF16, tag="v2")

            bqk_sb = consts.tile([128, 16], F32, tag="bqk")
            bp_sb = consts.tile([128, CT], F32, tag="bp")
            bv_sb = consts.tile([1, C], BF16, tag="bv")
            mask_sb = consts.tile([32, N], BF16, tag="binmask")
            ones_sb = consts.tile([128, 32], BF16, tag="ones")
            zbias_sb = consts.tile([128, 1], F32, tag="zbias")
            onesr_sb = consts.tile([1, 128], BF16, tag="onesr")
            sel2_sb = consts.tile([2, 128], BF16, tag="sel2")
            nc.sync.dma_start(out=sel2_sb[:], in_=sel2_ext[:])
            nc.sync.dma_start(out=bqk_sb[:], in_=bqk_ext[:])
            nc.sync.dma_start(out=bp_sb[:], in_=bp_ext[:])
            nc.sync.dma_start(out=bv_sb[:], in_=bv_ext[:])
            nc.sync.dma_start(out=mask_sb[:], in_=mask_ext[:])
            nc.vector.memset(ones_sb[:], 1.0)
            nc.vector.memset(zbias_sb[:], 0.0)
            nc.vector.memset(onesr_sb[:], 1.0)

            # per-half expt tiles, keyed for lagged use by PV/sums
            expt_tiles = {}   # half_id -> [expt_mt0, expt_mt1, expt_mt2]

            def sc_mt(i, mt):
                b, p = i // 8, i % 8
                pp = p % 4
                o = p
                moff, msize = M_TILES[mt]
                mb = (b % 4) * 32 if mt == 2 else 0
                if p % 4 == 0 and mt == 0:
                    expt_tiles[i // 4] = [
                        expt_pool.tile(
                            [128, 8, N], BF16, tag=f"expt{m}", name=f"expt{m}"
                        )
                        for m in range(3)
                    ]
                if mt == 0:
                    sc_mt.ps_s = pss_pool.tile(
                        [128, 2, 512], F32, tag="ps_s", name="ps_s"
                    )
                ps_s = sc_mt.ps_s
                for hh in range(2):
                    rb = 64 * hh
                    nc.tensor.matmul(
                        ps_s[mb : mb + msize, hh, 0:N],
                        lhsT=kt_sb[
                            rb : rb + 64,
                            o,
                            b * N + moff : b * N + moff + msize,
                        ],
                        rhs=qt_sb[rb : rb + 64, o, b * N : (b + 1) * N],
                        start=True,
                        stop=True,
                        tile_position=(rb, mb) if mt == 2 else None,
                    )
                nc.scalar.activation(
                    out=expt_tiles[i // 4][mt][
                        mb : mb + msize, 2 * pp : 2 * pp + 2, :
                    ],
                    in_=ps_s[mb : mb + msize, :, 0:N],
                    func=mybir.ActivationFunctionType.Exp,
                    bias=zbias_sb[0:msize, 0:1],
                    scale=SCALE,
                )

            def mask_op(i):
                pp = (i % 8) % 4
                nc.vector.tensor_tensor(
                    expt_tiles[i // 4][0][0:32, 2 * pp : 2 * pp + 2, :],
                    expt_tiles[i // 4][0][0:32, 2 * pp : 2 * pp + 2, :],
                    mask_sb[:, None, :].to_broadcast((32, 2, N)),
                    mybir.AluOpType.mult,
                )

            # ---------------- phase A: QKV projections ----------------
            with (
                tc.tile_pool(name="xa", bufs=1) as xa_pool,
                tc.tile_pool(name="wa", bufs=2) as wa_pool,
                tc.tile_pool(name="psA", bufs=4, space="PSUM") as psa_pool,
                tc.tile_pool(name="psAv", bufs=2, space="PSUM") as psav_pool,
            ):
                # prefetch the first weight tile BEFORE the bulky xt loads so
                # the first matmul doesn't queue ~20us behind 4.7MB of x DMA
                w_first = wa_pool.tile([128, CT, 128], BF16, tag="wqk", name="w_first")
                nc.sync.dma_start(out=w_first[:], in_=wqkt_r[:, :, 0:128])

                xt_sb = xa_pool.tile([128, CT, T], BF16, tag="xt")
                for o in range(CT):
                    nc.sync.dma_start(out=xt_sb[:, o, :], in_=xt_r[:, o, :])

                # q then k, transposed layout [cq, t]
                for proj in range(2):
                    dst = qt_sb if proj == 0 else kt_sb
                    for o in range(CT):
                        if proj == 0 and o == 0:
                            w_sb = w_first
                        else:
                            w_sb = wa_pool.tile(
                                [128, CT, 128], BF16, tag="wqk", name="w_sb"
                            )
                            j0 = proj * C + o * 128
                            nc.sync.dma_start(
                                out=w_sb[:], in_=wqkt_r[:, :, j0 : j0 + 128]
                            )
                        for c0 in range(0, T, 512):
                            csz = min(512, T - c0)
                            ps = psa_pool.tile([128, 512], F32, tag="psqk")
                            for kk in range(CT):
                                nc.tensor.matmul(
                                    ps[:, 0:csz],
                                    lhsT=w_sb[:, kk, :],
                                    rhs=xt_sb[:, kk, c0 : c0 + csz],
                                    start=(kk == 0),
                                    stop=(kk == CT - 1),
                                )
                            nc.vector.tensor_scalar(
                                out=dst[:, o, c0 : c0 + csz],
                                in0=ps[:, 0:csz],
                                scalar1=bqk_sb[:, proj * 8 + o : proj * 8 + o + 1],
                                scalar2=None,
                                op0=mybir.AluOpType.add,
                            )

                # contiguous staging of the 32-token mt2 tails, 4 batches
                # per 128-wide group (walrus: stationary AP needs 1 free dim)
                xg2_sb = xa_pool.tile([128, CT, 2, 128], BF16, tag="xg2")
                for kk in range(CT):
                    for g in range(2):
                        nc.vector.tensor_copy(
                            xg2_sb[:, kk, g, :],
                            xt_sb[:, kk, :].rearrange("p (b n) -> p b n", n=N)[
                                :, 4 * g : 4 * g + 4, 256:288
                            ],
                        )

                # v, natural layout [token, cv]
                for ch in range(2):
                    wv_sb = wa_pool.tile([128, CT, 512], BF16, tag="wv")
                    nc.sync.dma_start(
                        out=wv_sb[:], in_=wvt_r[:, :, ch * 512 : (ch + 1) * 512]
                    )
                    for b in range(BL):
                        for mt, (moff, msize) in enumerate(M_TILES[:2]):
                            t0 = b * N + moff
                            ps = psav_pool.tile([128, 512], F32, tag="psv")
                            for kk in range(CT):
                                nc.tensor.matmul(
                                    ps[:msize, :],
                                    lhsT=xt_sb[:, kk, t0 : t0 + msize],
                                    rhs=wv_sb[:, kk, :],
                                    start=(kk == 0),
                                    stop=False,
                                )
                            # bias row via rank-1 matmul (ones ⊗ bv)
                            nc.tensor.matmul(
                                ps[:msize, :],
                                lhsT=onesr_sb[0:1, 0:msize],
                                rhs=bv_sb[0:1, ch * 512 : (ch + 1) * 512],
                                start=False,
                                stop=True,
                            )
                            nc.scalar.copy(
                                out=v_sb[0:msize, b, mt, ch * 512 : (ch + 1) * 512],
                                in_=ps[:msize, :],
                            )
                    # mt2 (32-token tails): 4 batches packed on partitions
                    for g in range(2):
                        ps = psav_pool.tile([128, 512], F32, tag="psv")
                        for kk in range(CT):
                            nc.tensor.matmul(
                                ps[:],
                                lhsT=xg2_sb[:, kk, g, :],
                                rhs=wv_sb[:, kk, :],
                                start=(kk == 0),
                                stop=False,
                            )
                        nc.tensor.matmul(
                            ps[:],
                            lhsT=onesr_sb[0:1, 0:128],
                            rhs=bv_sb[0:1, ch * 512 : (ch + 1) * 512],
                            start=False,
                            stop=True,
                        )
                        for jj in range(4):
                            nc.scalar.copy(
                                out=v2_sb[
                                    32 * jj : 32 * jj + 32,
                                    g,
                                    ch * 512 : (ch + 1) * 512,
                                ],
                                in_=ps[32 * jj : 32 * jj + 32, :],
                            )

            # ---------------- phases B+C (global pair pipeline) ----------------
            # One "slot" per head-pair i (64 total).  At slot s we emit:
            #   scores+exp+mask for pair s (3 mt rounds, exp after each),
            #   PV for pair s-1 / sums for pair s-1 (fills the exp gaps),
            #   reciprocal for batch s//8-1 when s%8==0,
            #   bcast+normalize for pair s-9 (previous batch),
            #   out-projection chunk for pair s-10.
            # The 1-slot/9-slot lags keep the PE instruction stream dense so
            # the HAM clock gate stays at 8/8 (2.4 GHz) through phase B.
            with (
                tc.tile_pool(name="wpt", bufs=1) as wpt_pool,
                tc.tile_pool(name="outt", bufs=2) as outt_pool,
                tc.tile_pool(name="yc", bufs=3) as yc_pool,
                tc.tile_pool(name="expt", bufs=2) as expt_pool,
                tc.tile_pool(name="pvs", bufs=2) as pvs_pool,
                tc.tile_pool(name="sums", bufs=2) as sums_pool,
                tc.tile_pool(name="densep", bufs=10) as densep_pool,
                tc.tile_pool(name="psS", bufs=2, space="PSUM") as pss_pool,
                tc.tile_pool(name="psPV", bufs=1, space="PSUM") as pspv_pool,
                tc.tile_pool(name="psSum", bufs=1, space="PSUM") as pssum_pool,
                tc.tile_pool(name="psBC", bufs=1, space="PSUM") as psbc_pool,
                tc.tile_pool(name="psC", bufs=1, space="PSUM") as psc_pool,
            ):
                wpt_sb = wpt_pool.tile([128, CT, C], BF16, tag="wpt")
                for kk in range(CT):
                    nc.sync.dma_start(out=wpt_sb[:, kk, :], in_=wpt_r[:, kk, :])

                # per-batch staging, keyed for lagged use
                pvstage_t = {}    # batch -> tile
                sums_t = {}       # batch -> (sums_sb, sums_sr)
                dense_t = {}      # batch -> tile
                denseb_t = {}     # batch -> tile
                outt_t = {}       # batch -> tile

                def pv_block(i):
                    b, p = i // 8, i % 8
                    expt = expt_tiles[i // 4]
                    if p == 0:
                        pvstage_t[b] = pvs_pool.tile(
                            [128, 8, N], BF16, tag="pvstage", name="pvstage"
                        )
                    ps_pv = pspv_pool.tile([128, N], F32, tag="ps_pv")
                    pv_block.ps_pv = ps_pv
                    # mt-major: the two heads' matmuls hit distinct 64-col
                    # array strips (col tiling) and run concurrently.  Only
                    # the very first matmul carries start=True — its whole-
                    # bank has_written clear covers both head regions.
                    for mt, (moff, msize) in enumerate(M_TILES):
                        mb = (b % 4) * 32 if mt == 2 else 0
                        for hh in range(2):
                            h = 2 * p + hh
                            slot = h % 8
                            lhsT_v = (
                                v_sb[0:msize, b, mt, h * 64 : h * 64 + 64]
                                if mt < 2
                                else v2_sb[
                                    mb : mb + 32, b // 4, h * 64 : h * 64 + 64
                                ]
                            )
                            nc.tensor.matmul(
                                ps_pv[64 * hh : 64 * hh + 64, :],
                                lhsT=lhsT_v,
                                rhs=expt[mt][mb : mb + msize, slot, :],
                                start=(mt == 0),
                                stop=(mt == 2),
                                skip_group_check=True,
                                tile_position=((mb, 64 * hh) if mt == 2 else None),
                            )

                def sums_block(i):
                    b, p = i // 8, i % 8
                    expt = expt_tiles[i // 4]
                    if p == 0:
                        sums_sb = sums_pool.tile(
                            [128, 2, N], F32, tag="sums", name="sums"
                        )
                        sums_t[b] = (
                            sums_sb,
                            sums_sb.rearrange("(a c) s n -> a c s n", c=32),
                        )
                        dense_t[b] = sums_pool.tile(
                            [16, N], F32, tag="dense", name="dense"
                        )
                        denseb_t[b] = sums_pool.tile(
                            [16, N], BF16, tag="denseb", name="denseb"
                        )
                    sums_sb, sums_sr = sums_t[b]
                    ps_sm = pssum_pool.tile([128, N], F32, tag="ps_sm")
                    for mt, (moff, msize) in enumerate(M_TILES):
                        mb = (b % 4) * 32 if mt == 2 else 0
                        for hh in range(2):
                            h = 2 * p + hh
                            slot = h % 8
                            # ones [m, 32]: the column sum lands replicated on
                            # 32 partition rows so the later [0:33] copy reads
                            # no uninit PSUM
                            nc.tensor.matmul(
                                ps_sm[32 * hh : 32 * hh + 32, :],
                                lhsT=ones_sb[mb : mb + msize, :],
                                rhs=expt[mt][mb : mb + msize, slot, :],
                                start=(mt == 0),
                                stop=(mt == 2),
                                skip_group_check=True,
                                tile_position=((mb, 32 * hh) if mt == 2 else None),
                            )
                    # stage PV out of PSUM on the DVE (ACT is exp-bound)
                    nc.vector.tensor_copy(
                        pvstage_t[b][:, p, :], pv_block.ps_pv[:]
                    )
                    nc.vector.tensor_copy(sums_sb[0:33, p % 2, :], ps_sm[0:33, :])
                    nc.sync.dma_start(
                        out=dense_t[b][2 * p : 2 * p + 2, :],
                        in_=sums_sr[0:2, 0, p % 2, :],
                    )

                dp_t = {}  # pair -> [2, N] bf16 reciprocal row pair

                def recip_block(b):
                    nc.vector.reciprocal(out=dense_t[b][:], in_=dense_t[b][:])
                    nc.vector.tensor_copy(denseb_t[b][:], dense_t[b][:])
                    # prefetch ALL eight pairs' dp rows now so the
                    # bcast-matmul -> normalize chain never waits on DMA
                    # latency mid-slot (batch boundaries, kernel tail)
                    for p in range(8):
                        dp = densep_pool.tile([2, N], BF16, tag="dp", name="dp")
                        nc.sync.dma_start(
                            out=dp[:], in_=denseb_t[b][2 * p : 2 * p + 2, :]
                        )
                        dp_t[8 * b + p] = dp

                def bcast_norm_block(i):
                    b, p = i // 8, i % 8
                    if p == 0:
                        outt_t[b] = outt_pool.tile(
                            [128, CT, N], BF16, tag="outt_b", name="outt_b"
                        )
                    # broadcast via selector matmul: psbc[P,n] = dp[P//64,n]
                    psbc = psbc_pool.tile([128, N], F32, tag="psbc")
                    nc.tensor.matmul(
                        psbc[:],
                        lhsT=sel2_sb[:],
                        rhs=dp_t.pop(i)[:],
                        start=True,
                        stop=True,
                    )
                    nc.vector.tensor_tensor(
                        outt_t[b][:, p, :],
                        pvstage_t[b][:, p, :],
                        psbc[:],
                        mybir.AluOpType.mult,
                    )

                def proj_block(i):
                    b, o = i // 8, i % 8
                    ps = psc_pool.tile([128, N], F32, tag="psy", name="psy")
                    for kk in range(CT):
                        nc.tensor.matmul(
                            ps[:],
                            lhsT=wpt_sb[:, kk, o * 128 : (o + 1) * 128],
                            rhs=outt_t[b][:, kk, :],
                            start=(kk == 0),
                            stop=(kk == CT - 1),
                        )
                    y_sb = yc_pool.tile([128, N], F32, tag="y", name="y")
                    nc.vector.tensor_scalar(
                        out=y_sb[:],
                        in0=ps[:],
                        scalar1=bp_sb[:, o : o + 1],
                        scalar2=None,
                        op0=mybir.AluOpType.add,
                    )
                    nc.sync.dma_start(
                        out=out_r[:, o, b * N : (b + 1) * N], in_=y_sb[:]
                    )

                # proj lags 17 slots: chunk (b, o) reads ALL eight outt(b)
                # slices, and the last one is only EMITTED at slot 8b+16 —
                # emitting a read before its producing write is emitted gives
                # stale data (Tile orders reads only against prior writes).
                NP = BL * 8  # 64 pairs
                for s in range(NP + 17):
                    if s < NP:
                        sc_mt(s, 0)
                        if s >= 1:
                            pv_block(s - 1)
                        sc_mt(s, 1)
                        if s >= 1:
                            sums_block(s - 1)
                        if s >= 8 and s % 8 == 0:
                            recip_block(s // 8 - 1)
                        sc_mt(s, 2)
                        mask_op(s)
                        if s >= 9:
                            bcast_norm_block(s - 9)
                        if s >= 17:
                            proj_block(s - 17)
                    else:
                        if s - 1 < NP:
                            pv_block(s - 1)
                            sums_block(s - 1)
                        if s % 8 == 0 and s // 8 - 1 < BL:
                            recip_block(s // 8 - 1)
                        if s - 9 < NP:
                            bcast_norm_block(s - 9)
                        if s - 17 >= 0:
                            proj_block(s - 17)

    if split_waits:
        _split_multi_waits(nc)
    return nc


_NC_CACHE = None


def _get_nc():
    global _NC_CACHE
    if _NC_CACHE is None:
        _NC_CACHE = _build_nc()
    return _NC_CACHE


def _host_inputs(x, Wqkv, bqkv, Wproj, bproj):
    bf16 = ml_dtypes.bfloat16
    shared = {}
    shared["wqkt"] = np.ascontiguousarray(Wqkv[: 2 * C].T).astype(bf16)
    shared["wvt"] = np.ascontiguousarray(Wqkv[2 * C :].T).astype(bf16)
    shared["wpt"] = np.ascontiguousarray(Wproj.T).astype(bf16)
    shared["bv"] = bqkv[2 * C :].reshape(1, C).astype(bf16)
    shared["bqk"] = np.ascontiguousarray(
        bqkv[: 2 * C].reshape(2, 8, 128).transpose(2, 0, 1).reshape(128, 16)
    ).astype(np.float32)
    shared["bp"] = np.ascontiguousarray(bproj.reshape(CT, 128).T).astype(np.float32)
    sel2 = np.zeros((2, 128), bf16)
    sel2[0, 0:64] = 1.0
    sel2[1, 64:128] = 1.0
    shared["sel2"] = sel2
    m_ = np.arange(32)[:, None]
    n_ = np.arange(N)[None, :]
    shared["binmask"] = ((n_ < 32) & (n_ >= 4 * (m_ // 4))).astype(bf16)

    in_maps = []
    for i in range(8):
        xc = x[:, i * BL : (i + 1) * BL, :]  # (N, BL, C)
        xt = np.ascontiguousarray(xc.transpose(2, 1, 0).reshape(C, T)).astype(bf16)
        m = dict(shared)
        m["xt"] = xt
        in_maps.append(m)
    return in_maps


def kernel(x, Wqkv, bqkv, Wproj, bproj):
    x = np.asarray(x, dtype=np.float32)
    Wqkv = np.asarray(Wqkv, dtype=np.float32)
    bqkv = np.asarray(bqkv, dtype=np.float32)
    Wproj = np.asarray(Wproj, dtype=np.float32)
    bproj = np.asarray(bproj, dtype=np.float32)

    nc = _get_nc()
    in_maps = _host_inputs(x, Wqkv, bqkv, Wproj, bproj)
    res = run_bass_kernel_spmd(nc, in_maps, core_ids=list(range(8)))

    full = np.empty((N, 64, C), dtype=np.float32)
    for i in range(8):
        yT = np.asarray(res.results[i]["out"], dtype=np.float32)  # [C, T]
        full[:, i * BL : (i + 1) * BL, :] = yT.reshape(C, BL, N).transpose(2, 1, 0)
    return full



# revision 24
# speedup vs baseline: 1.2908x; 1.0756x over previous
"""Distributed Trainium2 Bass kernel for nn_Attention_69973607186925.

Multi-head attention (N=288 tokens, B=64 batch, C=1024, H=16 heads) with a
prompt-structured mask, data-parallel over batch across 8 NeuronCores
(8 batches = 128 heads per core, zero collectives).

Per-core dataflow (all matmuls bf16 -> f32 PSUM):
  phase A: QKV projections. q/k produced TRANSPOSED [c, token] (c on
           partitions) as scores operands; v produced NATURAL [token, c]
           as the PV stationary operand. Weights streamed, x resident.
  phase B: per (batch, head-pair): scoresT[m,n] = kT.T @ qT on the PE
           (keys m on partitions, queries n free), exp on ACT (scale 1/8
           folded in), prompt mask applied as a binary multiply on the
           first 32 key rows, PV = v.T @ expT accumulated into a
           pair-shared PSUM bank (odd head at column-position 64), column
           sums via ones-vector matmuls, reciprocal + cross-partition
           broadcast of 1/sum via a replicating SBUF->SBUF DMA, fused
           normalize-multiply into the transposed output [c, token].
  phase C: output projection from outT, bias added via per-partition
           scalar on the PSUM->SBUF copy, DMA to DRAM [1024, 2304].

Host side: shard batch, pre-transpose/pre-cast inputs (free), gather and
re-transpose the 8 per-core outputs.
"""

import sys

if "/opt/trn_rl_repo" not in sys.path:
    sys.path.insert(0, "/opt/trn_rl_repo")

import numpy as np
import ml_dtypes

import concourse.bass as bass
import concourse.mybir as mybir
import concourse.tile as tile
from concourse.bass_utils import run_bass_kernel_spmd

BF16 = mybir.dt.bfloat16
F32 = mybir.dt.float32

N = 288          # tokens per batch
BL = 8           # batches per core
C = 1024
H = 16           # heads per batch
HD = 64          # head dim
T = BL * N       # tokens per core (2304)
CT = C // 128    # c tiles (8)
NCH = T // N     # token chunks of 288 (8)
SCALE = HD ** -0.5
M_TILES = [(0, 128), (128, 128), (256, 32)]  # key tiles per batch


def _install_tile_drain_patch():
    """walrus in this container accepts only ONE semaphore wait per sync
    (SP) engine instruction; TileContext's final drain carries one wait
    per live semaphore.  Split them across single-wait nops (same engine,
    program order) before the drain."""
    from concourse.vector_clock import ScopedClock

    if getattr(tile.TileContext, "_drain_patch_installed", False):
        return

    def _drain_and_barrier_chunked(self, tick_clock, wait_clock):
        nc = self.nc
        collector = nc.sync.nop(nofuse=True, hint="drain_wait_collector")
        wait_clock.add_sem_waits(
            collector.ins, ScopedClock({None: tick_clock.global_clock})
        )
        si = collector.ins.sync_info
        waits = list(si.on_wait) if si and si.on_wait else []
        if len(waits) > 1:
            si.on_wait = waits[:1]
            for w in waits[1:]:
                extra = nc.sync.nop(nofuse=True, hint="drain_wait_chunk")
                esi = extra.ins.sync_info
                if esi is None:
                    extra.ins.sync_info = mybir.SyncInfo(on_wait=[w], on_update=[])
                else:
                    esi.on_wait = (esi.on_wait or []) + [w]
        nc.sync.drain()

        nc.all_engine_barrier()
        assert self.sems is not None
        popped = nc._tile_sem_poison_stack.pop()
        assert popped is self._sem_poison
        nc.clear_and_free_semaphores(list(self.sems.allocated().values()))
        nc.all_engine_barrier()

    tile.TileContext._drain_and_barrier = _drain_and_barrier_chunked
    tile.TileContext._drain_patch_installed = True


def _split_multi_waits(nc):
    """walrus in this container accepts only one semaphore wait per
    instruction.  For any instruction carrying N>1 waits, hoist N-1 of
    them onto same-engine NoOps placed immediately before it — engine
    program order makes this equivalent."""
    for fn in nc.m.functions:
        for blk in fn.blocks:
            insts = blk.instructions
            out = []
            changed = False
            for inst in insts:
                si = inst.sync_info
                if si is not None and si.on_wait and len(si.on_wait) > 1:
                    waits = list(si.on_wait)
                    for idx, w in enumerate(waits[:-1]):
                        out.append(
                            mybir.InstNoOp(
                                name=f"{inst.name}-hw{idx}",
                                engine=inst.engine,
                                ins=[],
                                outs=[],
                                bass_nofuse=True,
                                sync_info=mybir.SyncInfo(on_wait=[w], on_update=[]),
                            )
                        )
                    si.on_wait = [waits[-1]]
                    changed = True
                out.append(inst)
            if changed:
                insts[:] = out


def _build_nc(split_waits=True):
    _install_tile_drain_patch()
    nc = bass.Bass()

    xt_ext = nc.declare_dram_parameter("xt", [C, T], BF16, isOutput=False)
    wqkt_ext = nc.declare_dram_parameter("wqkt", [C, 2 * C], BF16, isOutput=False)
    wvt_ext = nc.declare_dram_parameter("wvt", [C, C], BF16, isOutput=False)
    wpt_ext = nc.declare_dram_parameter("wpt", [C, C], BF16, isOutput=False)
    bv_ext = nc.declare_dram_parameter("bv", [1, C], BF16, isOutput=False)
    bqk_ext = nc.declare_dram_parameter("bqk", [128, 16], F32, isOutput=False)
    bp_ext = nc.declare_dram_parameter("bp", [128, CT], F32, isOutput=False)
    mask_ext = nc.declare_dram_parameter("binmask", [32, N], BF16, isOutput=False)
    sel2_ext = nc.declare_dram_parameter("sel2", [2, 128], BF16, isOutput=False)
    out_ext = nc.declare_dram_parameter("out", [C, T], F32, isOutput=True)

    xt_r = xt_ext.rearrange("(o p) t -> p o t", p=128)
    wqkt_r = wqkt_ext.rearrange("(o p) j -> p o j", p=128)
    wvt_r = wvt_ext.rearrange("(o p) j -> p o j", p=128)
    wpt_r = wpt_ext.rearrange("(o p) j -> p o j", p=128)
    out_r = out_ext.rearrange("(o p) t -> p o t", p=128)

    with tile.TileContext(nc) as tc:
        with (
            tc.tile_pool(name="persist", bufs=1) as persist,
            tc.tile_pool(name="consts", bufs=1) as consts,
        ):
            qt_sb = persist.tile([128, CT, T], BF16, tag="qt")
            kt_sb = persist.tile([128, CT, T], BF16, tag="kt")
            v_sb = persist.tile([128, BL, 2, C], BF16, tag="v")
            v2_sb = persist.tile([128, 2, C], BF16, tag="v2")

            bqk_sb = consts.tile([128, 16], F32, tag="bqk")
            bp_sb = consts.tile([128, CT], F32, tag="bp")
            bv_sb = consts.tile([1, C], BF16, tag="bv")
            mask_sb = consts.tile([32, N], BF16, tag="binmask")
            ones_sb = consts.tile([128, 32], BF16, tag="ones")
            zbias_sb = consts.tile([128, 1], F32, tag="zbias")
            onesr_sb = consts.tile([1, 128], BF16, tag="onesr")
            sel2_sb = consts.tile([2, 128], BF16, tag="sel2")
            nc.sync.dma_start(out=sel2_sb[:], in_=sel2_ext[:])
            nc.sync.dma_start(out=bqk_sb[:], in_=bqk_ext[:])
            nc.sync.dma_start(out=bp_sb[:], in_=bp_ext[:])
            nc.sync.dma_start(out=bv_sb[:], in_=bv_ext[:])
            nc.sync.dma_start(out=mask_sb[:], in_=mask_ext[:])
            nc.vector.memset(ones_sb[:], 1.0)
            nc.vector.memset(zbias_sb[:], 0.0)
            nc.vector.memset(onesr_sb[:], 1.0)

            # per-half expt tiles, keyed for lagged use by PV/sums
            expt_tiles = {}   # half_id -> [expt_mt0, expt_mt1, expt_mt2]

            def sc_mt(i, mt):
                b, p = i // 8, i % 8
                pp = p % 4
                o = p
                moff, msize = M_TILES[mt]
                mb = (b % 4) * 32 if mt == 2 else 0
                if p % 4 == 0 and mt == 0:
                    expt_tiles[i // 4] = [
                        expt_pool.tile(
                            [128, 8, N], BF16, tag=f"expt{m}", name=f"expt{m}"
                        )
                        for m in range(3)
                    ]
                if mt == 0:
                    sc_mt.ps_s = pss_pool.tile(
                        [128, 2, 512], F32, tag="ps_s", name="ps_s"
                    )
                ps_s = sc_mt.ps_s
                for hh in range(2):
                    rb = 64 * hh
                    nc.tensor.matmul(
                        ps_s[mb : mb + msize, hh, 0:N],
                        lhsT=kt_sb[
                            rb : rb + 64,
                            o,
                            b * N + moff : b * N + moff + msize,
                        ],
                        rhs=qt_sb[rb : rb + 64, o, b * N : (b + 1) * N],
                        start=True,
                        stop=True,
                        tile_position=(rb, mb) if mt == 2 else None,
                    )
                nc.scalar.activation(
                    out=expt_tiles[i // 4][mt][
                        mb : mb + msize, 2 * pp : 2 * pp + 2, :
                    ],
                    in_=ps_s[mb : mb + msize, :, 0:N],
                    func=mybir.ActivationFunctionType.Exp,
                    bias=zbias_sb[0:msize, 0:1],
                    scale=SCALE,
                )

            def mask_op(i):
                pp = (i % 8) % 4
                nc.vector.tensor_tensor(
                    expt_tiles[i // 4][0][0:32, 2 * pp : 2 * pp + 2, :],
                    expt_tiles[i // 4][0][0:32, 2 * pp : 2 * pp + 2, :],
                    mask_sb[:, None, :].to_broadcast((32, 2, N)),
                    mybir.AluOpType.mult,
                )

            # ---------------- phase A: QKV projections ----------------
            with (
                tc.tile_pool(name="xa", bufs=1) as xa_pool,
                tc.tile_pool(name="wa", bufs=2) as wa_pool,
                tc.tile_pool(name="psA", bufs=4, space="PSUM") as psa_pool,
                tc.tile_pool(name="psAv", bufs=2, space="PSUM") as psav_pool,
            ):
                # prefetch the first weight tile BEFORE the bulky xt loads so
                # the first matmul doesn't queue ~20us behind 4.7MB of x DMA
                w_first = wa_pool.tile([128, CT, 128], BF16, tag="wqk", name="w_first")
                nc.sync.dma_start(out=w_first[:], in_=wqkt_r[:, :, 0:128])

                xt_sb = xa_pool.tile([128, CT, T], BF16, tag="xt")
                for o in range(CT):
                    nc.sync.dma_start(out=xt_sb[:, o, :], in_=xt_r[:, o, :])

                # q then k, transposed layout [cq, t]
                for proj in range(2):
                    dst = qt_sb if proj == 0 else kt_sb
                    for o in range(CT):
                        if proj == 0 and o == 0:
                            w_sb = w_first
                        else:
                            w_sb = wa_pool.tile(
                                [128, CT, 128], BF16, tag="wqk", name="w_sb"
                            )
                            j0 = proj * C + o * 128
                            nc.sync.dma_start(
                                out=w_sb[:], in_=wqkt_r[:, :, j0 : j0 + 128]
                            )
                        for c0 in range(0, T, 512):
                            csz = min(512, T - c0)
                            ps = psa_pool.tile([128, 512], F32, tag="psqk")
                            for kk in range(CT):
                                nc.tensor.matmul(
                                    ps[:, 0:csz],
                                    lhsT=w_sb[:, kk, :],
                                    rhs=xt_sb[:, kk, c0 : c0 + csz],
                                    start=(kk == 0),
                                    stop=(kk == CT - 1),
                                )
                            nc.vector.tensor_scalar(
                                out=dst[:, o, c0 : c0 + csz],
                                in0=ps[:, 0:csz],
                                scalar1=bqk_sb[:, proj * 8 + o : proj * 8 + o + 1],
                                scalar2=None,
                                op0=mybir.AluOpType.add,
                            )

                # contiguous staging of the 32-token mt2 tails, 4 batches
                # per 128-wide group (walrus: stationary AP needs 1 free dim)
                xg2_sb = xa_pool.tile([128, CT, 2, 128], BF16, tag="xg2")
                for kk in range(CT):
                    for g in range(2):
                        nc.vector.tensor_copy(
                            xg2_sb[:, kk, g, :],
                            xt_sb[:, kk, :].rearrange("p (b n) -> p b n", n=N)[
                                :, 4 * g : 4 * g + 4, 256:288
                            ],
                        )

                # v, natural layout [token, cv]
                for ch in range(2):
                    wv_sb = wa_pool.tile([128, CT, 512], BF16, tag="wv")
                    nc.sync.dma_start(
                        out=wv_sb[:], in_=wvt_r[:, :, ch * 512 : (ch + 1) * 512]
                    )
                    for b in range(BL):
                        for mt, (moff, msize) in enumerate(M_TILES[:2]):
                            t0 = b * N + moff
                            ps = psav_pool.tile([128, 512], F32, tag="psv")
                            for kk in range(CT):
                                nc.tensor.matmul(
                                    ps[:msize, :],
                                    lhsT=xt_sb[:, kk, t0 : t0 + msize],
                                    rhs=wv_sb[:, kk, :],
                                    start=(kk == 0),
                                    stop=False,
                                )
                            # bias row via rank-1 matmul (ones ⊗ bv)
                            nc.tensor.matmul(
                                ps[:msize, :],
                                lhsT=onesr_sb[0:1, 0:msize],
                                rhs=bv_sb[0:1, ch * 512 : (ch + 1) * 512],
                                start=False,
                                stop=True,
                            )
                            nc.scalar.copy(
                                out=v_sb[0:msize, b, mt, ch * 512 : (ch + 1) * 512],
                                in_=ps[:msize, :],
                            )
                    # mt2 (32-token tails): 4 batches packed on partitions
                    for g in range(2):
                        ps = psav_pool.tile([128, 512], F32, tag="psv")
                        for kk in range(CT):
                            nc.tensor.matmul(
                                ps[:],
                                lhsT=xg2_sb[:, kk, g, :],
                                rhs=wv_sb[:, kk, :],
                                start=(kk == 0),
                                stop=False,
                            )
                        nc.tensor.matmul(
                            ps[:],
                            lhsT=onesr_sb[0:1, 0:128],
                            rhs=bv_sb[0:1, ch * 512 : (ch + 1) * 512],
                            start=False,
                            stop=True,
                        )
                        for jj in range(4):
                            nc.scalar.copy(
                                out=v2_sb[
                                    32 * jj : 32 * jj + 32,
                                    g,
                                    ch * 512 : (ch + 1) * 512,
                                ],
                                in_=ps[32 * jj : 32 * jj + 32, :],
                            )

            # ---------------- phases B+C (global pair pipeline) ----------------
            # One "slot" per head-pair i (64 total).  At slot s we emit:
            #   scores+exp+mask for pair s (3 mt rounds, exp after each),
            #   PV for pair s-1 / sums for pair s-1 (fills the exp gaps),
            #   reciprocal for batch s//8-1 when s%8==0,
            #   bcast+normalize for pair s-9 (previous batch),
            #   out-projection chunk for pair s-10.
            # The 1-slot/9-slot lags keep the PE instruction stream dense so
            # the HAM clock gate stays at 8/8 (2.4 GHz) through phase B.
            with (
                tc.tile_pool(name="wpt", bufs=1) as wpt_pool,
                tc.tile_pool(name="outt", bufs=2) as outt_pool,
                tc.tile_pool(name="yc", bufs=3) as yc_pool,
                tc.tile_pool(name="expt", bufs=2) as expt_pool,
                tc.tile_pool(name="pvs", bufs=2) as pvs_pool,
                tc.tile_pool(name="sums", bufs=2) as sums_pool,
                tc.tile_pool(name="densep", bufs=10) as densep_pool,
                tc.tile_pool(name="psS", bufs=2, space="PSUM") as pss_pool,
                tc.tile_pool(name="psPV", bufs=1, space="PSUM") as pspv_pool,
                tc.tile_pool(name="psSum", bufs=1, space="PSUM") as pssum_pool,
                tc.tile_pool(name="psBC", bufs=1, space="PSUM") as psbc_pool,
                tc.tile_pool(name="psC", bufs=1, space="PSUM") as psc_pool,
            ):
                wpt_sb = wpt_pool.tile([128, CT, C], BF16, tag="wpt")
                for kk in range(CT):
                    nc.sync.dma_start(out=wpt_sb[:, kk, :], in_=wpt_r[:, kk, :])

                # per-batch staging, keyed for lagged use
                pvstage_t = {}    # batch -> tile
                sums_t = {}       # batch -> (sums_sb, sums_sr)
                dense_t = {}      # batch -> tile
                denseb_t = {}     # batch -> tile
                outt_t = {}       # batch -> tile

                def pv_block(i):
                    b, p = i // 8, i % 8
                    expt = expt_tiles[i // 4]
                    if p == 0:
                        pvstage_t[b] = pvs_pool.tile(
                            [128, 8, N], BF16, tag="pvstage", name="pvstage"
                        )
                    ps_pv = pspv_pool.tile([128, N], F32, tag="ps_pv")
                    pv_block.ps_pv = ps_pv
                    # mt-major: the two heads' matmuls hit distinct 64-col
                    # array strips (col tiling) and run concurrently.  Only
                    # the very first matmul carries start=True — its whole-
                    # bank has_written clear covers both head regions.
                    for mt, (moff, msize) in enumerate(M_TILES):
                        mb = (b % 4) * 32 if mt == 2 else 0
                        for hh in range(2):
                            h = 2 * p + hh
                            slot = h % 8
                            lhsT_v = (
                                v_sb[0:msize, b, mt, h * 64 : h * 64 + 64]
                                if mt < 2
                                else v2_sb[
                                    mb : mb + 32, b // 4, h * 64 : h * 64 + 64
                                ]
                            )
                            nc.tensor.matmul(
                                ps_pv[64 * hh : 64 * hh + 64, :],
                                lhsT=lhsT_v,
                                rhs=expt[mt][mb : mb + msize, slot, :],
                                start=(mt == 0),
                                stop=(mt == 2),
                                skip_group_check=True,
                                tile_position=((mb, 64 * hh) if mt == 2 else None),
                            )

                def sums_block(i):
                    b, p = i // 8, i % 8
                    expt = expt_tiles[i // 4]
                    if p == 0:
                        sums_sb = sums_pool.tile(
                            [128, 2, N], F32, tag="sums", name="sums"
                        )
                        sums_t[b] = (
                            sums_sb,
                            sums_sb.rearrange("(a c) s n -> a c s n", c=32),
                        )
                        dense_t[b] = sums_pool.tile(
                            [16, N], F32, tag="dense", name="dense"
                        )
                        denseb_t[b] = sums_pool.tile(
                            [16, N], BF16, tag="denseb", name="denseb"
                        )
                    sums_sb, sums_sr = sums_t[b]
                    ps_sm = pssum_pool.tile([128, N], F32, tag="ps_sm")
                    for mt, (moff, msize) in enumerate(M_TILES):
                        mb = (b % 4) * 32 if mt == 2 else 0
                        for hh in range(2):
                            h = 2 * p + hh
                            slot = h % 8
                            # ones [m, 32]: the column sum lands replicated on
                            # 32 partition rows so the later [0:33] copy reads
                            # no uninit PSUM
                            nc.tensor.matmul(
                                ps_sm[32 * hh : 32 * hh + 32, :],
                                lhsT=ones_sb[mb : mb + msize, :],
                                rhs=expt[mt][mb : mb + msize, slot, :],
                                start=(mt == 0),
                                stop=(mt == 2),
                                skip_group_check=True,
                                tile_position=((mb, 32 * hh) if mt == 2 else None),
                            )
                    # stage PV out of PSUM on the DVE (ACT is exp-bound)
                    nc.vector.tensor_copy(
                        pvstage_t[b][:, p, :], pv_block.ps_pv[:]
                    )
                    nc.vector.tensor_copy(sums_sb[0:33, p % 2, :], ps_sm[0:33, :])
                    nc.sync.dma_start(
                        out=dense_t[b][2 * p : 2 * p + 2, :],
                        in_=sums_sr[0:2, 0, p % 2, :],
                    )

                dp_t = {}  # pair -> [2, N] bf16 reciprocal row pair

                def recip_block(b):
                    nc.vector.reciprocal(out=dense_t[b][:], in_=dense_t[b][:])
                    nc.vector.tensor_copy(denseb_t[b][:], dense_t[b][:])
                    # prefetch ALL eight pairs' dp rows now so the
                    # bcast-matmul -> normalize chain never waits on DMA
                    # latency mid-slot (batch boundaries, kernel tail)
                    for p in range(8):
                        dp = densep_pool.tile([2, N], BF16, tag="dp", name="dp")
                        nc.sync.dma_start(
                            out=dp[:], in_=denseb_t[b][2 * p : 2 * p + 2, :]
                        )
                        dp_t[8 * b + p] = dp

                def bcast_norm_block(i):
                    b, p = i // 8, i % 8
                    if p == 0:
                        outt_t[b] = outt_pool.tile(
                            [128, CT, N], BF16, tag="outt_b", name="outt_b"
                        )
                    # broadcast via selector matmul: psbc[P,n] = dp[P//64,n]
                    psbc = psbc_pool.tile([128, N], F32, tag="psbc")
                    nc.tensor.matmul(
                        psbc[:],
                        lhsT=sel2_sb[:],
                        rhs=dp_t.pop(i)[:],
                        start=True,
                        stop=True,
                    )
                    nc.vector.tensor_tensor(
                        outt_t[b][:, p, :],
                        pvstage_t[b][:, p, :],
                        psbc[:],
                        mybir.AluOpType.mult,
                    )

                def proj_block(i, tail=False):
                    b, o = i // 8, i % 8
                    if tail:
                        # PV/sums banks are idle in the tail: round-robin the
                        # proj accumulators over three pools so chunks overlap
                        # 3-deep and the PE stream stays dense (HAM warm)
                        pool, tag = [
                            (psc_pool, "psy"),
                            (pspv_pool, "ps_pv"),
                            (pssum_pool, "ps_sm"),
                        ][i % 3]
                    else:
                        pool, tag = psc_pool, "psy"
                    ps = pool.tile([128, N], F32, tag=tag, name="psy")
                    for kk in range(CT):
                        nc.tensor.matmul(
                            ps[:],
                            lhsT=wpt_sb[:, kk, o * 128 : (o + 1) * 128],
                            rhs=outt_t[b][:, kk, :],
                            start=(kk == 0),
                            stop=(kk == CT - 1),
                        )
                    y_sb = yc_pool.tile([128, N], F32, tag="y", name="y")
                    nc.vector.tensor_scalar(
                        out=y_sb[:],
                        in0=ps[:],
                        scalar1=bp_sb[:, o : o + 1],
                        scalar2=None,
                        op0=mybir.AluOpType.add,
                    )
                    nc.sync.dma_start(
                        out=out_r[:, o, b * N : (b + 1) * N], in_=y_sb[:]
                    )

                # proj lags 17 slots: chunk (b, o) reads ALL eight outt(b)
                # slices, and the last one is only EMITTED at slot 8b+16 —
                # emitting a read before its producing write is emitted gives
                # stale data (Tile orders reads only against prior writes).
                NP = BL * 8  # 64 pairs
                for s in range(NP + 17):
                    if s < NP:
                        sc_mt(s, 0)
                        if s >= 1:
                            pv_block(s - 1)
                        sc_mt(s, 1)
                        if s >= 1:
                            sums_block(s - 1)
                        if s >= 8 and s % 8 == 0:
                            recip_block(s // 8 - 1)
                        sc_mt(s, 2)
                        mask_op(s)
                        if s >= 9:
                            bcast_norm_block(s - 9)
                        if s >= 17:
                            proj_block(s - 17)
                    else:
                        if s - 1 < NP:
                            pv_block(s - 1)
                            sums_block(s - 1)
                        if s % 8 == 0 and s // 8 - 1 < BL:
                            recip_block(s // 8 - 1)
                        if s - 9 < NP:
                            bcast_norm_block(s - 9)
                        if s - 17 >= 0:
                            proj_block(s - 17, tail=True)

    if split_waits:
        _split_multi_waits(nc)
    return nc


_NC_CACHE = None


def _get_nc():
    global _NC_CACHE
    if _NC_CACHE is None:
        _NC_CACHE = _build_nc()
    return _NC_CACHE


def _host_inputs(x, Wqkv, bqkv, Wproj, bproj):
    bf16 = ml_dtypes.bfloat16
    shared = {}
    shared["wqkt"] = np.ascontiguousarray(Wqkv[: 2 * C].T).astype(bf16)
    shared["wvt"] = np.ascontiguousarray(Wqkv[2 * C :].T).astype(bf16)
    shared["wpt"] = np.ascontiguousarray(Wproj.T).astype(bf16)
    shared["bv"] = bqkv[2 * C :].reshape(1, C).astype(bf16)
    shared["bqk"] = np.ascontiguousarray(
        bqkv[: 2 * C].reshape(2, 8, 128).transpose(2, 0, 1).reshape(128, 16)
    ).astype(np.float32)
    shared["bp"] = np.ascontiguousarray(bproj.reshape(CT, 128).T).astype(np.float32)
    sel2 = np.zeros((2, 128), bf16)
    sel2[0, 0:64] = 1.0
    sel2[1, 64:128] = 1.0
    shared["sel2"] = sel2
    m_ = np.arange(32)[:, None]
    n_ = np.arange(N)[None, :]
    shared["binmask"] = ((n_ < 32) & (n_ >= 4 * (m_ // 4))).astype(bf16)

    in_maps = []
    for i in range(8):
        xc = x[:, i * BL : (i + 1) * BL, :]  # (N, BL, C)
        xt = np.ascontiguousarray(xc.transpose(2, 1, 0).reshape(C, T)).astype(bf16)
        m = dict(shared)
        m["xt"] = xt
        in_maps.append(m)
    return in_maps


def kernel(x, Wqkv, bqkv, Wproj, bproj):
    x = np.asarray(x, dtype=np.float32)
    Wqkv = np.asarray(Wqkv, dtype=np.float32)
    bqkv = np.asarray(bqkv, dtype=np.float32)
    Wproj = np.asarray(Wproj, dtype=np.float32)
    bproj = np.asarray(bproj, dtype=np.float32)

    nc = _get_nc()
    in_maps = _host_inputs(x, Wqkv, bqkv, Wproj, bproj)
    res = run_bass_kernel_spmd(nc, in_maps, core_ids=list(range(8)))

    full = np.empty((N, 64, C), dtype=np.float32)
    for i in range(8):
        yT = np.asarray(res.results[i]["out"], dtype=np.float32)  # [C, T]
        full[:, i * BL : (i + 1) * BL, :] = yT.reshape(C, BL, N).transpose(2, 1, 0)
    return full

